# revision 29
# baseline (speedup 1.0000x reference)
"""FARGANSub Bass/Trainium2 kernel — data-parallel over 8 NeuronCores.

Layout strategy: activations live feature-major on-chip ([features x batch],
batch on the free dim, N=512 per batch-tile), so the whole 9-matmul chain runs
with zero inter-layer transposes. Weights are host-packed as [128, K_tiles, M]
f32r (full-rate fp32 mode on the PE). Natural-layout inputs are brought into
feature-major via the xbar DMA-transpose (2-byte only, so fp32 is shipped as
bf16 hi+lo pairs and reconstructed on-chip, bit-lossless to ~2^-17). Outputs
go back to natural layout via PE transposes. The pitch-window gather uses
indirect DMA with host-computed element offsets.
"""
import os
import sys
import types

import numpy as np
import ml_dtypes

sys.path.insert(0, "/opt/trn_rl_repo")
sys.path.insert(0, "/opt/pypackages")

import concourse.bass as bass
import concourse.tile as tile
import concourse.mybir as mybir

F32R = mybir.dt.float32r
F32 = mybir.dt.float32
BF16 = mybir.dt.bfloat16
I32 = mybir.dt.int32
AF = mybir.ActivationFunctionType
ALU = mybir.AluOpType

B = 8192
NCORES = 8
BC = B // NCORES          # rows per core (1024)
NT = 512                  # batch-tile (free dim)
NTILES = BC // NT         # 2 batch-tiles per core
SF = 40
CS = 256

PROFILE = False           # set by test harness for a traced run
_PROG_CACHE = {}


def _split_hilo(x):
    hi = x.astype(ml_dtypes.bfloat16)
    lo = (x - hi.astype(np.float32)).astype(ml_dtypes.bfloat16)
    return hi, lo


def _pack_w(wt, kpad=None):
    """W.T [in_f, out_f] -> [128, K, out_f] f32 packed K-tiles (zero padded)."""
    in_f, out_f = wt.shape
    k = (in_f + 127) // 128 if kpad is None else kpad
    buf = np.zeros((k * 128, out_f), dtype=np.float32)
    buf[:in_f] = wt
    return np.ascontiguousarray(buf.reshape(k, 128, out_f).transpose(1, 0, 2))


def _split_waits(nc, cap=1):
    """walrus here rejects >~1 sync-wait per instruction on several paths;
    split excess waits into chained NoOps on the same engine queue."""
    for f in nc.m.functions:
        for blk in f.blocks:
            out = []
            for inst in blk.instructions:
                si = inst.sync_info
                if si is not None and si.on_wait and len(si.on_wait) > cap:
                    waits = list(si.on_wait)
                    for w in waits[:-cap]:
                        out.append(mybir.InstNoOp(
                            name=nc.get_next_instruction_name(),
                            engine=inst.engine,
                            text_hint="waitsplit",
                            bass_nofuse=True,
                            sync_info=mybir.SyncInfo(on_wait=[w], on_update=[]),
                        ))
                    si.on_wait = waits[-cap:]
                out.append(inst)
            blk.instructions = out


def _build_program(split=True, debug=False, limit=5):
    nc = bass.Bass(trn_type="TRN2", target_bir_lowering=False, debug=False,
                   num_devices=NCORES)

    D = {}

    def din(name, shape, dt):
        D[name] = nc.dram_tensor(name, list(shape), dt, kind="ExternalInput").ap()
        return D[name]

    def dout(name, shape, dt=F32R):
        D[name] = nc.dram_tensor(name, list(shape), dt, kind="ExternalOutput").ap()
        return D[name]

    # activations
    din("xnat", (BC, 640), F32)   # [state3 | cond | phase]
    din("hnat", (BC, 768), F32)   # [state0 | state1 | state2]
    din("prev", (BC, SF), F32)
    din("gain", (BC, 1), F32)
    din("exc", (BC, 256), F32)
    din("offs", (BC, 1), I32)
    din("ebias", (1, 1), F32)          # exp(pgain_b)
    din("ident", (128, 128), F32R)
    din("ident32", (128, 128), F32)
    # weights (packed [128, K, M])
    din("w_fwc0", (128, 6 * 256), F32R)
    din("w_fglu", (128, 2 * 256), F32R)
    din("w_d2", (128, 2 * 256), F32R)
    din("w_dglu", (128, 2 * 256), F32R)
    for i in range(3):
        din(f"w_rz{i}", (128, 4 * 512), F32R)
        din(f"w_in{i}", (128, 2 * 256), F32R)
        din(f"w_hn{i}", (128, 2 * 256), F32R)
        din(f"w_glu{i}", (128, 2 * 256), F32R)
    din("w_out", (128, 8 * 41), F32R)
    # outputs
    dout("sig", (BC, SF), F32)
    if debug:
        dout("dbg_fpre", (128, 1024)); dout("dbg_fo", (128, 1024))
        dout("dbg_d2o", (128, 1024)); dout("dbg_rz", (128, 2048))
        dout("dbg_n", (128, 1024)); dout("dbg_hnew", (128, 1024))
        dout("dbg_xcat", (128, 3072)); dout("dbg_hcat", (128, 3072))
    dout("excnew", (BC, 256), F32)
    dout("gru1", (BC, CS), F32); dout("gru2", (BC, CS), F32); dout("gru3", (BC, CS), F32)
    dout("fst", (BC, 480), F32)

    exc_flat = D["exc"].rearrange("a b -> (a b)").rearrange("a -> a ()")

    from contextlib import ExitStack
    with tile.TileContext(nc) as tc, ExitStack() as ctx:
        wp = ctx.enter_context(tc.tile_pool(name="wp", bufs=1))
        stg = ctx.enter_context(tc.tile_pool(name="stg", bufs=1))
        big = ctx.enter_context(tc.tile_pool(name="big", bufs=1))
        mid = ctx.enter_context(tc.tile_pool(name="mid", bufs=1))
        mid2 = ctx.enter_context(tc.tile_pool(name="mid2", bufs=1))
        sm = ctx.enter_context(tc.tile_pool(name="sm", bufs=4))
        psd = ctx.enter_context(tc.tile_pool(name="psd", bufs=6, space="PSUM"))
        pst = ctx.enter_context(tc.tile_pool(name="pst", bufs=1, space="PSUM"))

        # ---- constants first (first transposes need them) ----
        ident = wp.tile([128, 128], F32R, tag="ident")
        nc.sync.dma_start(ident[:], D["ident"][:, :])
        ident32 = wp.tile([128, 128], F32, tag="ident32")
        nc.sync.dma_start(ident32[:], D["ident32"][:, :])
        ebt = wp.tile([128, 1], F32, tag="ebt")
        eb_bcast = bass.AP(tensor=D["ebias"].tensor, offset=0, ap=[[0, 128], [1, 1]])
        nc.gpsimd.dma_start(ebt[:], eb_bcast)

        # ---- weights to SBUF (once) ----
        W = {}
        for name, kk, mm in [("w_fwc0", 6, 256), ("w_fglu", 2, 256),
                             ("w_d2", 2, 256), ("w_dglu", 2, 256),
                             ("w_rz0", 4, 512), ("w_in0", 2, 256), ("w_hn0", 2, 256), ("w_glu0", 2, 256),
                             ("w_rz1", 4, 512), ("w_in1", 2, 256), ("w_hn1", 2, 256), ("w_glu1", 2, 256),
                             ("w_rz2", 4, 512), ("w_in2", 2, 256), ("w_hn2", 2, 256), ("w_glu2", 2, 256),
                             ("w_out", 8, 41)]:
            W[name] = wp.tile([128, kk, mm], F32R, tag=name, name=name)
            nc.sync.dma_start(W[name][:], D[name].rearrange("p (k m) -> p k m", k=kk))
        AFT, AFS = AF.Tanh, AF.Sigmoid

        XC, HC, TPN, GN = {}, {}, {}, {}

        def stageA(t):
            r0 = t * NT

            # ---- stage A: feature-major inputs via PE transposes ----
            xcat = big.tile([128, 6, NT], F32R, tag="xcat", bufs=1)
            hcat = big.tile([128, 6, NT], F32R, tag="hcat", bufs=2)
            for c in range(4):
                rc = slice(r0 + 128 * c, r0 + 128 * (c + 1))
                xn = stg.tile([128, 640], F32, tag="stgx")
                nc.sync.dma_start(xn[:], D["xnat"][rc, :])
                ps_x = pst.tile([128, 1024], F32, tag="tr")
                for j in range(5):
                    nc.tensor.transpose(ps_x[:, 128 * j:128 * (j + 1)],
                                        xn[:, 128 * j:128 * (j + 1)], ident32[:])
                nc.vector.tensor_copy(xcat[:, 0:5, 128 * c:128 * (c + 1)],
                                      ps_x.rearrange("p (j n) -> p j n", j=8)[:, 0:5, :])
                hn = stg.tile([128, 768], F32, tag="stgh")
                nc.sync.dma_start(hn[:], D["hnat"][rc, :])
                ps_h = pst.tile([128, 1024], F32, tag="tr")
                for j in range(6):
                    nc.tensor.transpose(ps_h[:, 128 * j:128 * (j + 1)],
                                        hn[:, 128 * j:128 * (j + 1)], ident32[:])
                nc.scalar.copy(hcat[:, 0:6, 128 * c:128 * (c + 1)],
                               ps_h.rearrange("p (j n) -> p j n", j=8)[:, 0:6, :])

            XC[t], HC[t] = xcat, hcat

        def stageB(t):
            r0 = t * NT
            xcat = XC[t]

            # ---- stage B: natural-layout gather/scale + transpose into xcat K5 ----
            ps_tp = pst.tile([128, 1024], F32, tag="tr")
            gaints, tpnats = [], []
            for c in range(4):
                rc = slice(r0 + 128 * c, r0 + 128 * (c + 1))
                offs_sb = sm.tile([128, 1], I32, tag="offs")
                nc.sync.dma_start(offs_sb[:], D["offs"][rc, :])
                fp = sm.tile([128, SF], F32, tag="fp")
                nc.gpsimd.indirect_dma_start(
                    out=fp[:], out_offset=None, in_=exc_flat,
                    in_offset=bass.IndirectOffsetOnAxis(ap=offs_sb[:, :1], axis=0))
                prevt = sm.tile([128, SF], F32, tag="prevt")
                nc.sync.dma_start(prevt[:], D["prev"][rc, :])
                gaint = sm.tile([128, 1], F32, tag="gaint", bufs=8)
                nc.sync.dma_start(gaint[:], D["gain"][rc, :])
                gtmp = sm.tile([128, 1], F32, tag="gtmp")
                nc.vector.tensor_scalar_add(gtmp[:], gaint[:], 1e-5)
                rgain = sm.tile([128, 1], F32, tag="rgain")
                nc.vector.reciprocal(rgain[:], gtmp[:])
                tpnat = sm.tile([128, 80], F32, tag="tpnat", bufs=8)
                nc.vector.tensor_scalar_mul(tpnat[:, 0:SF], fp[:], rgain[:, :1])
                nc.vector.tensor_scalar_mul(tpnat[:, SF:80], prevt[:], rgain[:, :1])
                nc.sync.dma_start(D["fst"][rc, 320:400], tpnat[:])
                nc.tensor.transpose(ps_tp[0:80, 128 * c:128 * (c + 1)], tpnat[:], ident32[:])
                gaints.append(gaint)
                tpnats.append(tpnat)
            nc.vector.tensor_copy(xcat[0:80, 5, :], ps_tp[0:80, 0:512])
            TPN[t], GN[t] = tpnats, gaints

        def stageCDEF(t):
            r0 = t * NT
            xcat, hcat = XC[t], HC[t]
            tpnats, gaints = TPN[t], GN[t]
            yield

            if debug and t == 0:
                nc.sync.dma_start(D["dbg_xcat"][:, :], xcat.rearrange("p k n -> p (k n)"))
                nc.sync.dma_start(D["dbg_hcat"][:, :], hcat.rearrange("p k n -> p (k n)"))
            if limit < 2:
                return
            # ---- stage C: fwc0 ----
            fpre = mid.tile([128, 2, NT], F32R, tag="fpre")
            for m in range(2):
                ps_f = psd.tile([128, 512], F32, tag="half", name="ps_f")
                for k in range(6):
                    p = 80 if k == 5 else 128
                    nc.tensor.matmul(
                        ps_f[:, 0:512],
                        W["w_fwc0"][0:p, k, 128 * m:128 * (m + 1)],
                        xcat[0:p, k, :], start=(k == 0), stop=(k == 5))
                nc.scalar.activation(fpre[:, m, :], ps_f[:, 0:512], AFT)
            sgt = mid.tile([128, 2, NT], F32R, tag="sigtmp", bufs=2, name="sgt")
            for m in range(2):
                ps_g = psd.tile([128, 512], F32, tag="half", name="ps_g")
                for k in range(2):
                    nc.tensor.matmul(
                        ps_g[:, 0:512],
                        W["w_fglu"][:, k, 128 * m:128 * (m + 1)],
                        fpre[:, k, :], start=(k == 0), stop=(k == 1))
                nc.scalar.activation(sgt[:, m, :], ps_g[:, 0:512], AFS)
            fo = mid.tile([128, 2, NT], F32R, tag="fo", bufs=2)
            nc.vector.tensor_mul(fo.rearrange("p k n -> p (k n)"),
                                 fpre.rearrange("p k n -> p (k n)"),
                                 sgt.rearrange("p k n -> p (k n)"))

            if debug and t == 0:
                nc.sync.dma_start(D["dbg_fpre"][:, :], fpre.rearrange("p k n -> p (k n)"))
                nc.sync.dma_start(D["dbg_fo"][:, :], fo.rearrange("p k n -> p (k n)"))

            yield
            # ---- stage D: dense2 ----
            dpre = mid.tile([128, 2, NT], F32R, tag="dpre")
            for m in range(2):
                ps_d = psd.tile([128, 512], F32, tag="half", name="ps_d")
                for k in range(2):
                    nc.tensor.matmul(
                        ps_d[:, 0:512],
                        W["w_d2"][:, k, 128 * m:128 * (m + 1)],
                        fo[:, k, :], start=(k == 0), stop=(k == 1))
                nc.scalar.activation(dpre[:, m, :], ps_d[:, 0:512], AFT)
            dsg = mid.tile([128, 2, NT], F32R, tag="sigtmp", bufs=2, name="dsg")
            for m in range(2):
                ps_dg = psd.tile([128, 512], F32, tag="half", name="ps_dg")
                for k in range(2):
                    nc.tensor.matmul(
                        ps_dg[:, 0:512],
                        W["w_dglu"][:, k, 128 * m:128 * (m + 1)],
                        dpre[:, k, :], start=(k == 0), stop=(k == 1))
                nc.scalar.activation(dsg[:, m, :], ps_dg[:, 0:512], AFS)
            d2o = mid.tile([128, 2, NT], F32R, tag="d2o", bufs=2)
            nc.vector.tensor_mul(d2o.rearrange("p k n -> p (k n)"),
                                 dpre.rearrange("p k n -> p (k n)"),
                                 dsg.rearrange("p k n -> p (k n)"))

            if debug and t == 0:
                nc.sync.dma_start(D["dbg_d2o"][:, :], d2o.rearrange("p k n -> p (k n)"))

            yield
            # ---- stage E: GRUs ----
            if limit < 3:
                return
            xin_gru = d2o
            gru_outs = []
            for i in range(3):
                hK = [hcat[:, 2 * i, :], hcat[:, 2 * i + 1, :]]
                xK = [xin_gru[:, 0, :], xin_gru[:, 1, :]]
                rz = mid.tile([128, 2048], F32R, tag="rz", bufs=2)
                for m in range(4):
                    ps_q = psd.tile([128, 512], F32, tag="half", name="ps_q")
                    for k in range(4):
                        rhs = xK[k] if k < 2 else hK[k - 2]
                        nc.tensor.matmul(
                            ps_q[:, 0:512],
                            W[f"w_rz{i}"][:, k, 128 * m:128 * (m + 1)],
                            rhs, start=(k == 0), stop=(k == 3))
                    nc.scalar.activation(rz[:, 512 * m:512 * (m + 1)], ps_q[:, 0:512], AFS)
                hfm = hcat.rearrange("p k n -> p (k n)")[:, 1024 * i:1024 * (i + 1)]
                ntile = mid.tile([128, 1024], F32R, tag="ntile", bufs=2)
                hnew = mid.tile([128, 2, NT], F32R, tag="hnew", bufs=2, name=f"hnew{i}")
                for m in range(2):
                    sl = slice(512 * m, 512 * (m + 1))
                    ps_a = psd.tile([128, 512], F32, tag="half", name="ps_a")
                    for k in range(2):
                        nc.tensor.matmul(
                            ps_a[:, 0:512],
                            W[f"w_hn{i}"][:, k, 128 * m:128 * (m + 1)],
                            hK[k], start=(k == 0), stop=(k == 1))
                    tn = mid.tile([128, 512], F32R, tag="ttmp", bufs=4, name="tn")
                    nc.vector.tensor_mul(tn[:], rz[:, sl], ps_a[:, 0:512])
                    ps_b = psd.tile([128, 512], F32, tag="half", name="ps_b")
                    for k in range(2):
                        nc.tensor.matmul(
                            ps_b[:, 0:512],
                            W[f"w_in{i}"][:, k, 128 * m:128 * (m + 1)],
                            xK[k], start=(k == 0), stop=(k == 1))
                    t2 = mid.tile([128, 512], F32R, tag="ttmp", bufs=4, name="t2")
                    nc.vector.tensor_add(t2[:], tn[:], ps_b[:, 0:512])
                    nc.scalar.activation(ntile[:, sl], t2[:], AFT)
                    dtile = mid.tile([128, 512], F32R, tag="ttmp", bufs=4, name="dtile")
                    nc.vector.tensor_sub(dtile[:], hfm[:, sl], ntile[:, sl])
                    zd = mid.tile([128, 512], F32R, tag="ttmp", bufs=4, name="zd")
                    nc.vector.tensor_mul(zd[:], rz[:, 1024 + 512 * m:1024 + 512 * (m + 1)], dtile[:])
                    nc.vector.tensor_add(hnew[:, m, :], ntile[:, sl], zd[:])
                if debug and t == 0 and i == 0:
                    nc.sync.dma_start(D["dbg_rz"][:, :], rz[:, 0:2048])
                    nc.sync.dma_start(D["dbg_n"][:, :], ntile[:])
                if debug and t == 0 and i == 0:
                    nc.sync.dma_start(D["dbg_hnew"][:, :], hnew.rearrange("p k n -> p (k n)"))
                # output transposes -> natural
                if limit >= 4:
                    ps_tr = pst.tile([128, 1024], F32R, tag="tr")
                    for ft in range(2):
                        for c in range(4):
                            nc.tensor.transpose(
                                ps_tr[:, 256 * c + 128 * ft:256 * c + 128 * (ft + 1)],
                                hnew[:, ft, 128 * c:128 * (c + 1)], ident[:])
                    hnat = mid2.tile([128, 4, 256], F32, tag="hnat")
                    nc.scalar.copy(hnat.rearrange("p k n -> p (k n)"), ps_tr[:])
                    for c in range(4):
                        rc = slice(r0 + 128 * c, r0 + 128 * (c + 1))
                        nc.sync.dma_start(D[f"gru{i + 1}"][rc, :], hnat[:, c, :])
                # GLU
                gs = mid.tile([128, 1024], F32R, tag="sigtmp", bufs=2, name="gs")
                for m in range(2):
                    ps_gl = psd.tile([128, 512], F32, tag="half", name="ps_gl")
                    for k in range(2):
                        nc.tensor.matmul(
                            ps_gl[:, 0:512],
                            W[f"w_glu{i}"][:, k, 128 * m:128 * (m + 1)],
                            hnew[:, k, :], start=(k == 0), stop=(k == 1))
                    nc.scalar.activation(gs[:, 512 * m:512 * (m + 1)], ps_gl[:, 0:512], AFS)
                go = mid.tile([128, 2, NT], F32R, tag=f"go{i}", bufs=2)
                nc.vector.tensor_mul(go.rearrange("p k n -> p (k n)"),
                                     hnew.rearrange("p k n -> p (k n)"), gs[:])
                gru_outs.append(go)
                xin_gru = go
                yield

            if limit < 5:
                return
            # ---- stage F: sig_out + pitch gain ----
            featK = [gru_outs[0][:, 0, :], gru_outs[0][:, 1, :],
                     gru_outs[1][:, 0, :], gru_outs[1][:, 1, :],
                     gru_outs[2][:, 0, :], gru_outs[2][:, 1, :],
                     d2o[:, 0, :], d2o[:, 1, :]]
            ps_s = psd.tile([128, 512], F32, tag="half", name="ps_s")
            for k in range(8):
                nc.tensor.matmul(ps_s[0:41, 0:512], W["w_out"][:, k, :], featK[k],
                                 start=(k == 0), stop=(k == 7))
            sgp = mid.tile([128, NT], F32, tag="sgp")
            nc.scalar.copy(sgp[0:41, :], ps_s[0:41, 0:512])
            ps_st = pst.tile([128, 1024], F32, tag="tr")
            for c in range(4):
                nc.tensor.transpose(ps_st[:, 41 * c:41 * (c + 1)],
                                    sgp[0:41, 128 * c:128 * (c + 1)], ident32[0:41, 0:41])
            for c in range(4):
                rc = slice(r0 + 128 * c, r0 + 128 * (c + 1))
                sgn = sm.tile([128, SF], F32, tag="sgn")
                nc.scalar.activation(sgn[:], ps_st[:, 41 * c:41 * c + SF], AFT)
                tcolt = sm.tile([128, 1], F32, tag="tcolt")
                nc.scalar.activation(tcolt[:], ps_st[:, 41 * c + SF:41 * c + 41],
                                     AFT, scale=0.5)
                pa = sm.tile([128, 1], F32, tag="pa")
                nc.vector.tensor_scalar(pa[:], tcolt[:], ebt[:, :1], ebt[:, :1],
                                        op0=ALU.mult, op1=ALU.add)
                pb = sm.tile([128, 1], F32, tag="pb")
                nc.vector.tensor_scalar(pb[:], tcolt[:], -1.0, 1.0,
                                        op0=ALU.mult, op1=ALU.add)
                prb = sm.tile([128, 1], F32, tag="prb")
                nc.vector.reciprocal(prb[:], pb[:])
                pg = sm.tile([128, 1], F32, tag="pg")
                nc.vector.tensor_mul(pg[:], pa[:], prb[:])
                m1 = sm.tile([128, SF], F32, tag="m1")
                nc.vector.tensor_scalar_mul(m1[:], tpnats[c][:, 0:SF], pg[:, :1])
                m2 = sm.tile([128, SF], F32, tag="m2")
                nc.vector.tensor_add(m2[:], m1[:], sgn[:])
                signat = sm.tile([128, SF], F32, tag="signat")
                nc.vector.tensor_scalar_mul(signat[:], m2[:], gaints[c][:, :1])
                nc.sync.dma_start(D["sig"][rc, :], signat[:])
                nc.sync.dma_start(D["excnew"][rc, 216:256], signat[:])

        stageA(0)
        stageB(0)
        load_weights()
        stageA(1)
        stageB(1)
        gens = [stageCDEF(0), stageCDEF(1)]
        done = [False, False]
        # stagger: advance tile 0 two stages so the tiles' serial chains offset
        for _ in range(3):
            next(gens[0])
        while not all(done):
            for gi, g in enumerate(gens):
                if not done[gi]:
                    try:
                        next(g)
                    except StopIteration:
                        done[gi] = True

        # ---- whole-slab DRAM->DRAM copies (independent; lowest priority) ----
        nc.sync.dma_start(D["excnew"][:, 0:216], D["exc"][:, 40:256])
        nc.sync.dma_start(D["fst"][:, 0:320], D["xnat"][:, 240:560])
        nc.sync.dma_start(D["fst"][:, 400:480], D["xnat"][:, 560:640])

    if split:
        _split_waits(nc, cap=1)
    return nc
    return nc


def _host_prep(inputs):
    """Shard + pack inputs for the 8 cores. Returns in_maps list."""
    f32 = np.float32
    cond = np.asarray(inputs["cond"], f32)
    prev = np.asarray(inputs["prev"], f32)
    exc = np.asarray(inputs["exc_mem"], f32)
    phase = np.asarray(inputs["phase"], f32)
    period = np.asarray(inputs["period"])
    st0 = np.asarray(inputs["state0"], f32)
    st1 = np.asarray(inputs["state1"], f32)
    st2 = np.asarray(inputs["state2"], f32)
    st3 = np.asarray(inputs["state3"], f32)
    gain = np.asarray(inputs["gain"], f32)

    # weights (shared)
    fwc0_w = np.asarray(inputs["fwc0_w"], f32)
    perm = np.r_[0:480, 480:560, 640:720, 560:600, 600:640]
    shared = {
        "w_fwc0": _pack_w(fwc0_w[:, perm].T).reshape(128, -1),
        "w_fglu": _pack_w(np.asarray(inputs["fwc0_glu_w"], f32).T).reshape(128, -1),
        "w_d2": _pack_w(np.asarray(inputs["d2_w"], f32).T).reshape(128, -1),
        "w_dglu": _pack_w(np.asarray(inputs["d2_glu_w"], f32).T).reshape(128, -1),
        "w_out": _pack_w(np.concatenate(
            [np.asarray(inputs["sig_out_w"], f32),
             np.asarray(inputs["pgain_w"], f32)], axis=0).T).reshape(128, -1),
        "ident": np.eye(128, dtype=f32),
        "ident32": np.eye(128, dtype=f32),
        "ebias": np.exp(np.asarray(inputs["pgain_b"], f32)).reshape(1, 1),
    }
    for i, nm in enumerate(["gru1", "gru2", "gru3"]):
        wih = np.asarray(inputs[f"{nm}_wih"], f32)
        whh = np.asarray(inputs[f"{nm}_whh"], f32)
        wrz = np.concatenate([wih[0:512].T, whh[0:512].T], axis=0)  # [512, 512]
        shared[f"w_rz{i}"] = _pack_w(wrz).reshape(128, -1)
        shared[f"w_in{i}"] = _pack_w(wih[512:768].T).reshape(128, -1)
        shared[f"w_hn{i}"] = _pack_w(whh[512:768].T).reshape(128, -1)
        shared[f"w_glu{i}"] = _pack_w(np.asarray(inputs[f"{nm}_glu_w"], f32).T).reshape(128, -1)

    pclip = np.clip(period.astype(np.int64), SF + 2, 254)
    s_off = (256 - pclip).astype(np.int32)  # window start within each row

    in_maps = []
    for cidx in range(NCORES):
        R = slice(cidx * BC, (cidx + 1) * BC)
        offs = (np.arange(BC, dtype=np.int32) * 256 + s_off[R]).reshape(BC, 1)
        m = dict(shared)
        m.update({
            "xnat": np.ascontiguousarray(
                np.concatenate([st3[R], cond[R], phase[R]], axis=1)),
            "hnat": np.ascontiguousarray(
                np.concatenate([st0[R], st1[R], st2[R]], axis=1)),
            "prev": np.ascontiguousarray(prev[R]),
            "gain": np.ascontiguousarray(gain[R]),
            "exc": np.ascontiguousarray(exc[R]),
            "offs": offs,
        })
        in_maps.append(m)
    return in_maps


def kernel(**inputs):
    from concourse import bass_utils

    if "nc" not in _PROG_CACHE:
        _PROG_CACHE["nc"] = _build_program()
    nc = _PROG_CACHE["nc"]

    in_maps = _host_prep(inputs)

    trace = bool(PROFILE)
    if trace and "antenv.axon_hooks" not in sys.modules:
        try:
            from trn_agent_boot.trn_boot import _ntff_profile_via_ctypes
            _m = types.ModuleType("antenv.axon_hooks")
            _m.get_axon_ntff_profile_hook = lambda: _ntff_profile_via_ctypes(
                "/opt/axon/libaxon_pjrt.so")
            _m.set_axon_ntff_profile_hook = lambda h: None
            sys.modules["antenv.axon_hooks"] = _m
        except Exception:
            trace = False

    res = bass_utils.run_bass_kernel_spmd(
        nc, in_maps, core_ids=list(range(NCORES)), trace=trace)
    kernel.last_result = res

    def cat(name):
        return np.concatenate([np.asarray(res.results[c][name], np.float32)
                               for c in range(NCORES)], axis=0)

    sig = cat("sig")
    excnew = cat("excnew")
    g1, g2, g3 = cat("gru1"), cat("gru2"), cat("gru3")
    fst = cat("fst")
    return sig, excnew, g1, g2, g3, fst
